# revision 12
# baseline (speedup 1.0000x reference)
"""Trainium2 Bass kernel for an int4-quantized DeepseekMLP (gate/up/down + SiLU).

Strategy (8 NeuronCores, tensor-parallel over the intermediate dim):
  - Each core owns a slice of the 11008 intermediate rows (6x1408 + 2x1280,
    padded to a uniform 1408 with zero-scale rows so all cores run one NEFF).
  - On device, per core:
      * x [4096, 4096] fp32 is cast to bf16 (DRAM->DRAM cast DMA, column
        chunks so the x^T transposes can start early).
      * int4 codes (host-unpacked to uint8) are dequantized on the DVE with two
        tensor_tensor ops per 128-row tile (subtract zero, multiply scale) using
        step-0 broadcast APs over the per-group scale/zero vectors.
      * Dequantized weights are transposed ONCE through the DMA xbar into a
        W^T DRAM scratch; the main loop re-reads them with plain DMAs.
      * The three matmuls run on the PE with everything in transposed layout
        (contraction dim on partitions). g^T/u^T accumulate in PSUM; SiLU runs
        on the scalar engine straight from PSUM; h^T = silu(g^T)*u^T on the
        DVE feeds the down matmul.
      * Partial down outputs (out^T) are ReduceScattered (bf16) over the 8
        cores along the output-feature dim, one collective per token block.
  - Host reassembles the full [4, 1024, 4096] fp32 output from the 8 shards.

HWDGE ring discipline (FIFO per issuing engine, so emission order == service
order): the ACT ring carries x^T xbar transposes + up-strip loads; the SP
(sync) ring carries weight-dequant xbar transposes + gate/down strip loads,
with the down transposes emitted after gateup_0 so tb0's gate strips aren't
stuck behind them. Everything else (codes, stores, casts, collectives) runs
on the gpsimd SWDGE path.
"""

import os

import numpy as np

import concourse.bass as bass
import concourse.mybir as mybir
import concourse.tile as tile
from concourse.tile import add_dep_helper
from concourse import bacc
import concourse.bass_utils as bass_utils

N_CORES = 8
B, S = 4, 1024
T = B * S            # 4096 tokens
H = 4096             # hidden
INTER = 11008
ISL = 1408           # per-core inter slice (padded)
G = 64               # quant group size
TB = 1024            # token block
NTB = T // TB        # 4
HT = H // 128        # 32 k-tiles for gate/up
IT = ISL // 128      # 11 i-tiles
NGH = H // G         # 64 groups along hidden (gate/up)
DG = ISL // G        # 22 groups along inter slice (down)
QH = 256             # down ho-slab height
NQ = H // QH         # 16 slabs

CORE_SIZES = [1408] * 6 + [1280] * 2

dt = mybir.dt
Alu = mybir.AluOpType

LAST_RESULTS = None


def _build():
    nc = bacc.Bacc("TRN2", target_bir_lowering=False, debug=False,
                   num_devices=N_CORES)

    x = nc.dram_tensor("x", [T, H], dt.float32, kind="ExternalInput")
    gc = nc.dram_tensor("gc", [ISL, H], dt.uint8, kind="ExternalInput")
    uc = nc.dram_tensor("uc", [ISL, H], dt.uint8, kind="ExternalInput")
    dc = nc.dram_tensor("dc", [H, ISL], dt.uint8, kind="ExternalInput")
    gs = nc.dram_tensor("gs", [ISL, NGH], dt.float32, kind="ExternalInput")
    gz = nc.dram_tensor("gz", [ISL, NGH], dt.float32, kind="ExternalInput")
    us = nc.dram_tensor("us", [ISL, NGH], dt.float32, kind="ExternalInput")
    uz = nc.dram_tensor("uz", [ISL, NGH], dt.float32, kind="ExternalInput")
    dsc = nc.dram_tensor("dsc", [H, DG], dt.float32, kind="ExternalInput")
    dzr = nc.dram_tensor("dzr", [H, DG], dt.float32, kind="ExternalInput")
    outT = nc.dram_tensor("outT", [H // N_CORES, T], dt.float32,
                          kind="ExternalOutput")

    with tile.TileContext(nc) as tc:
        with (
            tc.tile_pool(name="dram", bufs=1, space="DRAM") as dram,
            tc.tile_pool(name="xt", bufs=1) as xt_pool,
            tc.tile_pool(name="hp", bufs=2) as h_pool,
            tc.tile_pool(name="stage", bufs=3) as st_pool,   # dequant/xprep staging
            tc.tile_pool(name="wstream", bufs=5) as w_pool,  # main-loop gu strips
            tc.tile_pool(name="dstream", bufs=2) as d_pool,  # main-loop down strips
            tc.tile_pool(name="codes", bufs=2) as c_pool,
            tc.tile_pool(name="sz", bufs=4) as sz_pool,
            tc.tile_pool(name="act", bufs=2) as a_pool,
            tc.tile_pool(name="ob", bufs=4) as o_pool,
            tc.tile_pool(name="psgu", bufs=1, space="PSUM") as ps_gu,
            tc.tile_pool(name="psd", bufs=4, space="PSUM") as ps_d,
        ):
            # ---- x^T prep: load fp32 rows, cast to bf16 on the DVE, then
            # xbar-transpose SBUF->SBUF straight into the xT tile.
            def make_xT(tb):
                xT = xt_pool.tile([128, HT, TB], dt.bfloat16, tag="xT",
                                  name=f"xT_{tb}")
                with nc.named_scope(f"xT_{tb}"):
                    for rt in range(TB // 128):
                        rows = slice(tb * TB + rt * 128, tb * TB + (rt + 1) * 128)
                        for hf in range(2):
                            hsl = bass.ts(hf, H // 2)
                            xrow = st_pool.tile([128, H // 2], dt.float32,
                                                tag="stage",
                                                name=f"xrow_{tb}_{rt}_{hf}")
                            nc.gpsimd.dma_start(xrow[:], x[rows, hsl])
                            xrbf = st_pool.tile([128, H // 2], dt.bfloat16,
                                                tag="stage",
                                                name=f"xrbf_{tb}_{rt}_{hf}")
                            nc.vector.tensor_copy(xrbf[:], xrow[:])
                            nc.scalar.dma_start(
                                xT[:, hf * (HT // 2):(hf + 1) * (HT // 2),
                                   rt * 128:(rt + 1) * 128],
                                xrbf[:],
                                transpose=True,
                            )
                return xT

            # ---- dequant weights + transpose-once into W^T DRAM
            def dequant_rows(codes_dram, s_dram, z_dram, it, width, ngroups, tag):
                """One 128-row tile: (codes - zero) * scale with broadcast APs."""
                cs = c_pool.tile([128, width], dt.uint8, tag="codes",
                                 name=f"cs_{tag}")
                nc.gpsimd.dma_start(cs[:], codes_dram[it * 128:(it + 1) * 128, :])
                ssb = sz_pool.tile([128, ngroups], dt.float32, tag="ssb",
                                   name=f"ssb_{tag}")
                zsb = sz_pool.tile([128, ngroups], dt.float32, tag="zsb",
                                   name=f"zsb_{tag}")
                nc.gpsimd.dma_start(ssb[:], s_dram[it * 128:(it + 1) * 128, :])
                nc.gpsimd.dma_start(zsb[:], z_dram[it * 128:(it + 1) * 128, :])
                tmp = st_pool.tile([128, width], dt.bfloat16, tag="stage",
                                   name=f"tmp_{tag}")
                wb = st_pool.tile([128, width], dt.bfloat16, tag="stage",
                                  name=f"wb_{tag}")
                nc.vector.tensor_tensor(
                    tmp.rearrange("p (g k) -> p g k", k=G),
                    cs.rearrange("p (g k) -> p g k", k=G),
                    zsb[:, :, None].broadcast_to([128, ngroups, G]),
                    op=Alu.subtract,
                )
                nc.vector.tensor_tensor(
                    wb.rearrange("p (g k) -> p g k", k=G),
                    tmp.rearrange("p (g k) -> p g k", k=G),
                    ssb[:, :, None].broadcast_to([128, ngroups, G]),
                    op=Alu.mult,
                )
                return wb

            # gate/up: dequant [128, H] rows, xbar-transpose straight from SBUF
            # into a [128, HT, 128] strip, store to W^T DRAM for plain re-reads.
            gT_dram, uT_dram = [], []

            def dequant_gu_tile(it, nm, codes_d, s_d, z_d, lst):
                wb = dequant_rows(codes_d, s_d, z_d, it, H, NGH, f"{nm}{it}")
                wTs = st_pool.tile([128, HT, 128], dt.bfloat16,
                                   tag="stage", name=f"wTs_{nm}{it}")
                nc.sync.dma_start(wTs[:, :, :], wb[:], transpose=True)
                wT_d = dram.tile([128, HT * 128], dt.bfloat16,
                                 tag=f"{nm}T{it}", name=f"{nm}T{it}")
                nc.gpsimd.dma_start(wT_d[:], wTs.rearrange("p a b -> p (a b)"))
                lst.append(wT_d)

            # it=0 strips first so the PE can start as soon as x^T lands
            with nc.named_scope("dequant_gu"):
                dequant_gu_tile(0, "g", gc, gs, gz, gT_dram)
                dequant_gu_tile(0, "u", uc, us, uz, uT_dram)

            xT0 = make_xT(0)

            with nc.named_scope("dequant_gu"):
                for it in range(1, IT):
                    dequant_gu_tile(it, "g", gc, gs, gz, gT_dram)
                    dequant_gu_tile(it, "u", uc, us, uz, uT_dram)

            # down: dequant [128, ISL] rows into slab DRAM tiles (natural layout)
            d_nat = [dram.tile([QH, ISL], dt.bfloat16, tag=f"dnat{q}",
                               name=f"dnat{q}")
                     for q in range(NQ)]
            with nc.named_scope("dequant_d"):
                for ot in range(H // 128):
                    wb = dequant_rows(dc, dsc, dzr, ot, ISL, DG, f"d{ot}")
                    q, r = divmod(ot, QH // 128)
                    nc.gpsimd.dma_start(d_nat[q][r * 128:(r + 1) * 128, :], wb[:])

            dT_dram = [dram.tile([128, IT * QH], dt.bfloat16, tag=f"dT{q}",
                                 name=f"dT{q}")
                       for q in range(NQ)]

            def emit_transpose_d():
                # one 3D-dest xbar transpose per down slab into W^T DRAM:
                # dest[p, it, j] = d_nat[q][j, it*128 + p]
                with nc.named_scope("transpose_d"):
                    for q in range(NQ):
                        dTs = st_pool.tile([128, IT, QH], dt.bfloat16,
                                           tag="stage", name=f"dTs_{q}")
                        nc.scalar.dma_start(dTs[:, :, :], d_nat[q][:, :],
                                          transpose=True)
                        nc.gpsimd.dma_start(
                            dT_dram[q][:], dTs.rearrange("p a b -> p (a b)"))

            # ---- main loop over token blocks
            for tb in range(NTB):
                xT = xT0 if tb == 0 else make_xT(tb)

                h3 = h_pool.tile([128, IT, TB], dt.bfloat16, tag="h3",
                                 name=f"h3_{tb}")
                with nc.named_scope(f"gateup_{tb}"):
                    for it in range(IT):
                        wgT = w_pool.tile([128, HT, 128], dt.bfloat16,
                                          tag="wstrip", name=f"wgT_{tb}_{it}")
                        wuT = w_pool.tile([128, HT, 128], dt.bfloat16,
                                          tag="wstrip", name=f"wuT_{tb}_{it}")
                        nc.sync.dma_start(
                            wgT.rearrange("p a b -> p (a b)"), gT_dram[it][:])
                        nc.gpsimd.dma_start(
                            wuT.rearrange("p a b -> p (a b)"), uT_dram[it][:])

                        gps = ps_gu.tile([128, TB], dt.float32, tag="gps",
                                         name=f"gps_{tb}_{it}")
                        ups = ps_gu.tile([128, TB], dt.float32, tag="ups",
                                         name=f"ups_{tb}_{it}")
                        for n in range(TB // 512):
                            nsl = bass.ts(n, 512)
                            for ht in range(HT):
                                nc.tensor.matmul(
                                    gps[:, nsl],
                                    wgT[:, ht, :],
                                    xT[:, ht, nsl],
                                    start=(ht == 0), stop=(ht == HT - 1),
                                )
                            for ht in range(HT):
                                last_up_mm = nc.tensor.matmul(
                                    ups[:, nsl],
                                    wuT[:, ht, :],
                                    xT[:, ht, nsl],
                                    start=(ht == 0), stop=(ht == HT - 1),
                                )
                        sil = a_pool.tile([128, TB], dt.bfloat16, tag="sil",
                                          name=f"sil_{tb}_{it}")
                        nc.scalar.activation(sil[:], gps[:],
                                             mybir.ActivationFunctionType.Silu)
                        nc.vector.tensor_tensor(h3[:, it, :], sil[:], ups[:],
                                                op=Alu.mult)

                if tb == 0:
                    # down xbar transposes go on the sync ring after tb0's gate
                    # strip loads so those aren't FIFO-blocked behind them.
                    emit_transpose_d()

                first_down_mm = [None]
                with nc.named_scope(f"down_{tb}"):
                    for n in range(TB // 512):
                        nsl = bass.ts(n, 512)
                        part = dram.tile([H, 512], dt.bfloat16,
                                         tag=f"part{tb}_{n}",
                                         name=f"part{tb}_{n}")
                        for q in range(NQ):
                            wdT = d_pool.tile([128, IT, QH], dt.bfloat16,
                                              tag="dstrip",
                                              name=f"wdT_{tb}_{n}_{q}")
                            nc.gpsimd.dma_start(
                                wdT.rearrange("p a b -> p (a b)"), dT_dram[q][:])
                            for ho in range(QH // 128):
                                dps = ps_d.tile([128, 512], dt.float32,
                                                tag="dps",
                                                name=f"dps_{tb}_{n}_{q}_{ho}")
                                for it in range(IT):
                                    mm = nc.tensor.matmul(
                                        dps[:],
                                        wdT[:, it, ho * 128:(ho + 1) * 128],
                                        h3[:, it, nsl],
                                        start=(it == 0), stop=(it == IT - 1),
                                    )
                                    if first_down_mm[0] is None:
                                        first_down_mm[0] = mm
                                        # keep the down block after this tb's
                                        # gate/up matmuls in the PE stream
                                        add_dep_helper(
                                            mm.ins, last_up_mm.ins, sync=False,
                                            reason="down after gateup")
                                ob = o_pool.tile([128, 512], dt.bfloat16,
                                                 tag="ob",
                                                 name=f"ob_{tb}_{n}_{q}_{ho}")
                                nc.scalar.copy(ob[:], dps[:])
                                nc.gpsimd.dma_start(
                                    part[(q * (QH // 128) + ho) * 128:
                                         (q * (QH // 128) + ho + 1) * 128, :],
                                    ob[:],
                                )

                        rs_o = dram.tile([H // N_CORES, 512], dt.bfloat16,
                                         tag=f"rs{tb}_{n}", name=f"rs{tb}_{n}")
                        nc.gpsimd.collective_compute(
                            "ReduceScatter",
                            Alu.add,
                            replica_groups=[list(range(N_CORES))],
                            ins=[part.opt()],
                            outs=[rs_o.opt()],
                        )
                        nc.gpsimd.dma_start(
                            outT[:, tb * TB + n * 512: tb * TB + (n + 1) * 512],
                            rs_o[:])

    nc.compile()
    return nc


def _unpack_codes(Wq):
    """int32 [out, in/2] holding 0..255 byte values -> uint8 codes [out, in].
    Column 2j is the high nibble of byte j, column 2j+1 the low nibble."""
    b = Wq.astype(np.uint8)
    codes = np.empty((Wq.shape[0], Wq.shape[1] * 2), np.uint8)
    codes[:, 0::2] = (b >> 4) & 0xF
    codes[:, 1::2] = b & 0xF
    return codes


def _pad_rows(a, n):
    if a.shape[0] == n:
        return np.ascontiguousarray(a)
    pad = np.zeros((n - a.shape[0],) + a.shape[1:], a.dtype)
    return np.ascontiguousarray(np.concatenate([a, pad], axis=0))


def _pad_cols(a, n):
    if a.shape[1] == n:
        return np.ascontiguousarray(a)
    pad = np.zeros((a.shape[0], n - a.shape[1]), a.dtype)
    return np.ascontiguousarray(np.concatenate([a, pad], axis=1))


def kernel(x, gate_Wq, up_Wq, down_Wq, gate_scale, gate_zero,
           up_scale, up_zero, down_scale, down_zero):
    global LAST_RESULTS

    x2 = np.ascontiguousarray(np.asarray(x, np.float32).reshape(T, H))
    g_codes = _unpack_codes(np.asarray(gate_Wq))
    u_codes = _unpack_codes(np.asarray(up_Wq))
    d_codes = _unpack_codes(np.asarray(down_Wq))

    starts = np.cumsum([0] + CORE_SIZES)
    in_maps = []
    for c in range(N_CORES):
        lo, hi = int(starts[c]), int(starts[c + 1])
        glo, ghi = lo // G, hi // G
        in_maps.append({
            "x": x2,
            "gc": _pad_rows(g_codes[lo:hi], ISL),
            "uc": _pad_rows(u_codes[lo:hi], ISL),
            "dc": _pad_cols(d_codes[:, lo:hi], ISL),
            "gs": _pad_rows(np.asarray(gate_scale, np.float32)[lo:hi], ISL),
            "gz": _pad_rows(np.asarray(gate_zero, np.float32)[lo:hi], ISL),
            "us": _pad_rows(np.asarray(up_scale, np.float32)[lo:hi], ISL),
            "uz": _pad_rows(np.asarray(up_zero, np.float32)[lo:hi], ISL),
            "dsc": _pad_cols(np.asarray(down_scale, np.float32)[:, glo:ghi], DG),
            "dzr": _pad_cols(np.asarray(down_zero, np.float32)[:, glo:ghi], DG),
        })

    nc = _build()

    trace = os.environ.get("KERNEL_TRACE", "0") == "1"
    kw = {}
    if trace:
        kw = dict(trace=True, trace_cores=[0])
    res = bass_utils.run_bass_kernel_spmd(
        nc, in_maps, core_ids=list(range(N_CORES)), **kw)
    LAST_RESULTS = res

    out = np.empty((T, H), np.float32)
    shard = H // N_CORES
    for c in range(N_CORES):
        out[:, c * shard:(c + 1) * shard] = res.results[c]["outT"].T
    return out.reshape(B, S, H)


# revision 13
# speedup vs baseline: 1.0821x; 1.0821x over previous
"""Trainium2 Bass kernel for an int4-quantized DeepseekMLP (gate/up/down + SiLU).

Strategy (8 NeuronCores, tensor-parallel over the intermediate dim):
  - Each core owns a slice of the 11008 intermediate rows (6x1408 + 2x1280,
    padded to a uniform 1408 with zero-scale rows so all cores run one NEFF).
  - On device, per core:
      * x [4096, 4096] fp32 is cast to bf16 (DRAM->DRAM cast DMA, column
        chunks so the x^T transposes can start early).
      * int4 codes (host-unpacked to uint8) are dequantized on the DVE with two
        tensor_tensor ops per 128-row tile (subtract zero, multiply scale) using
        step-0 broadcast APs over the per-group scale/zero vectors.
      * Dequantized weights are transposed ONCE through the DMA xbar into a
        W^T DRAM scratch; the main loop re-reads them with plain DMAs.
      * The three matmuls run on the PE with everything in transposed layout
        (contraction dim on partitions). g^T/u^T accumulate in PSUM; SiLU runs
        on the scalar engine straight from PSUM; h^T = silu(g^T)*u^T on the
        DVE feeds the down matmul.
      * Partial down outputs (out^T) are ReduceScattered (bf16) over the 8
        cores along the output-feature dim, one collective per token block.
  - Host reassembles the full [4, 1024, 4096] fp32 output from the 8 shards.

HWDGE ring discipline (FIFO per issuing engine, so emission order == service
order): the ACT ring carries x^T xbar transposes + up-strip loads; the SP
(sync) ring carries weight-dequant xbar transposes + gate/down strip loads,
with the down transposes emitted after gateup_0 so tb0's gate strips aren't
stuck behind them. Everything else (codes, stores, casts, collectives) runs
on the gpsimd SWDGE path.
"""

import os

import numpy as np

import concourse.bass as bass
import concourse.mybir as mybir
import concourse.tile as tile
from concourse.tile import add_dep_helper
from concourse import bacc
import concourse.bass_utils as bass_utils

N_CORES = 8
B, S = 4, 1024
T = B * S            # 4096 tokens
H = 4096             # hidden
INTER = 11008
ISL = 1408           # per-core inter slice (padded)
G = 64               # quant group size
TB = 1024            # token block
NTB = T // TB        # 4
HT = H // 128        # 32 k-tiles for gate/up
IT = ISL // 128      # 11 i-tiles
NGH = H // G         # 64 groups along hidden (gate/up)
DG = ISL // G        # 22 groups along inter slice (down)
QH = 256             # down ho-slab height
NQ = H // QH         # 16 slabs

CORE_SIZES = [1408] * 6 + [1280] * 2

dt = mybir.dt
Alu = mybir.AluOpType

LAST_RESULTS = None


def _build():
    nc = bacc.Bacc("TRN2", target_bir_lowering=False, debug=False,
                   num_devices=N_CORES)

    x = nc.dram_tensor("x", [T, H], dt.float32, kind="ExternalInput")
    gc = nc.dram_tensor("gc", [ISL, H], dt.uint8, kind="ExternalInput")
    uc = nc.dram_tensor("uc", [ISL, H], dt.uint8, kind="ExternalInput")
    dc = nc.dram_tensor("dc", [H, ISL], dt.uint8, kind="ExternalInput")
    gs = nc.dram_tensor("gs", [ISL, NGH], dt.float32, kind="ExternalInput")
    gz = nc.dram_tensor("gz", [ISL, NGH], dt.float32, kind="ExternalInput")
    us = nc.dram_tensor("us", [ISL, NGH], dt.float32, kind="ExternalInput")
    uz = nc.dram_tensor("uz", [ISL, NGH], dt.float32, kind="ExternalInput")
    dsc = nc.dram_tensor("dsc", [H, DG], dt.float32, kind="ExternalInput")
    dzr = nc.dram_tensor("dzr", [H, DG], dt.float32, kind="ExternalInput")
    outT = nc.dram_tensor("outT", [H // N_CORES, T], dt.float32,
                          kind="ExternalOutput")

    with tile.TileContext(nc) as tc:
        with (
            tc.tile_pool(name="dram", bufs=1, space="DRAM") as dram,
            tc.tile_pool(name="xt", bufs=1) as xt_pool,
            tc.tile_pool(name="hp", bufs=2) as h_pool,
            tc.tile_pool(name="stage", bufs=3) as st_pool,   # dequant/xprep staging
            tc.tile_pool(name="wstream", bufs=5) as w_pool,  # main-loop gu strips
            tc.tile_pool(name="dstream", bufs=2) as d_pool,  # main-loop down strips
            tc.tile_pool(name="codes", bufs=2) as c_pool,
            tc.tile_pool(name="sz", bufs=4) as sz_pool,
            tc.tile_pool(name="act", bufs=2) as a_pool,
            tc.tile_pool(name="ob", bufs=2) as o_pool,
            tc.tile_pool(name="psgu", bufs=1, space="PSUM") as ps_gu,
            tc.tile_pool(name="psd", bufs=2, space="PSUM") as ps_d,
        ):
            # ---- x^T prep: load fp32 rows, cast to bf16 on the DVE, then
            # xbar-transpose SBUF->SBUF straight into the xT tile.
            def make_xT(tb):
                xT = xt_pool.tile([128, HT, TB], dt.bfloat16, tag="xT",
                                  name=f"xT_{tb}")
                with nc.named_scope(f"xT_{tb}"):
                    for rt in range(TB // 128):
                        rows = slice(tb * TB + rt * 128, tb * TB + (rt + 1) * 128)
                        for hf in range(2):
                            hsl = bass.ts(hf, H // 2)
                            xrow = st_pool.tile([128, H // 2], dt.float32,
                                                tag="stage",
                                                name=f"xrow_{tb}_{rt}_{hf}")
                            nc.sync.dma_start(xrow[:], x[rows, hsl])
                            xrbf = st_pool.tile([128, H // 2], dt.bfloat16,
                                                tag="stage",
                                                name=f"xrbf_{tb}_{rt}_{hf}")
                            nc.vector.tensor_copy(xrbf[:], xrow[:])
                            nc.scalar.dma_start(
                                xT[:, hf * (HT // 2):(hf + 1) * (HT // 2),
                                   rt * 128:(rt + 1) * 128],
                                xrbf[:],
                                transpose=True,
                            )
                return xT

            # ---- dequant weights + transpose-once into W^T DRAM
            def dequant_rows(codes_dram, s_dram, z_dram, it, width, ngroups, tag):
                """One 128-row tile: (codes - zero) * scale with broadcast APs."""
                cs = c_pool.tile([128, width], dt.uint8, tag="codes",
                                 name=f"cs_{tag}")
                nc.gpsimd.dma_start(cs[:], codes_dram[it * 128:(it + 1) * 128, :])
                ssb = sz_pool.tile([128, ngroups], dt.float32, tag="ssb",
                                   name=f"ssb_{tag}")
                zsb = sz_pool.tile([128, ngroups], dt.float32, tag="zsb",
                                   name=f"zsb_{tag}")
                nc.gpsimd.dma_start(ssb[:], s_dram[it * 128:(it + 1) * 128, :])
                nc.gpsimd.dma_start(zsb[:], z_dram[it * 128:(it + 1) * 128, :])
                tmp = st_pool.tile([128, width], dt.bfloat16, tag="stage",
                                   name=f"tmp_{tag}")
                wb = st_pool.tile([128, width], dt.bfloat16, tag="stage",
                                  name=f"wb_{tag}")
                nc.vector.tensor_tensor(
                    tmp.rearrange("p (g k) -> p g k", k=G),
                    cs.rearrange("p (g k) -> p g k", k=G),
                    zsb[:, :, None].broadcast_to([128, ngroups, G]),
                    op=Alu.subtract,
                )
                nc.vector.tensor_tensor(
                    wb.rearrange("p (g k) -> p g k", k=G),
                    tmp.rearrange("p (g k) -> p g k", k=G),
                    ssb[:, :, None].broadcast_to([128, ngroups, G]),
                    op=Alu.mult,
                )
                return wb

            # gate/up: dequant [128, H] rows, xbar-transpose straight from SBUF
            # into a [128, HT, 128] strip, store to W^T DRAM for plain re-reads.
            gT_dram, uT_dram = [], []

            def dequant_gu_tile(it, nm, codes_d, s_d, z_d, lst):
                wb = dequant_rows(codes_d, s_d, z_d, it, H, NGH, f"{nm}{it}")
                wTs = st_pool.tile([128, HT, 128], dt.bfloat16,
                                   tag="stage", name=f"wTs_{nm}{it}")
                nc.sync.dma_start(wTs[:, :, :], wb[:], transpose=True)
                wT_d = dram.tile([128, HT * 128], dt.bfloat16,
                                 tag=f"{nm}T{it}", name=f"{nm}T{it}")
                nc.gpsimd.dma_start(wT_d[:], wTs.rearrange("p a b -> p (a b)"))
                lst.append(wT_d)

            # it=0 strips first so the PE can start as soon as x^T lands
            with nc.named_scope("dequant_gu"):
                dequant_gu_tile(0, "g", gc, gs, gz, gT_dram)
                dequant_gu_tile(0, "u", uc, us, uz, uT_dram)

            xT0 = make_xT(0)

            with nc.named_scope("dequant_gu"):
                for it in range(1, IT):
                    dequant_gu_tile(it, "g", gc, gs, gz, gT_dram)
                    dequant_gu_tile(it, "u", uc, us, uz, uT_dram)

            # down: dequant [128, ISL] rows into slab DRAM tiles (natural layout)
            d_nat = [dram.tile([QH, ISL], dt.bfloat16, tag=f"dnat{q}",
                               name=f"dnat{q}")
                     for q in range(NQ)]
            with nc.named_scope("dequant_d"):
                for ot in range(H // 128):
                    wb = dequant_rows(dc, dsc, dzr, ot, ISL, DG, f"d{ot}")
                    q, r = divmod(ot, QH // 128)
                    nc.gpsimd.dma_start(d_nat[q][r * 128:(r + 1) * 128, :], wb[:])

            dT_dram = [dram.tile([128, IT * QH], dt.bfloat16, tag=f"dT{q}",
                                 name=f"dT{q}")
                       for q in range(NQ)]

            def emit_transpose_d():
                # one 3D-dest xbar transpose per down slab into W^T DRAM:
                # dest[p, it, j] = d_nat[q][j, it*128 + p]
                with nc.named_scope("transpose_d"):
                    for q in range(NQ):
                        dTs = st_pool.tile([128, IT, QH], dt.bfloat16,
                                           tag="stage", name=f"dTs_{q}")
                        nc.scalar.dma_start(dTs[:, :, :], d_nat[q][:, :],
                                          transpose=True)
                        nc.gpsimd.dma_start(
                            dT_dram[q][:], dTs.rearrange("p a b -> p (a b)"))

            # ---- main loop over token blocks
            for tb in range(NTB):
                xT = xT0 if tb == 0 else make_xT(tb)

                h3 = h_pool.tile([128, IT, TB], dt.bfloat16, tag="h3",
                                 name=f"h3_{tb}")
                with nc.named_scope(f"gateup_{tb}"):
                    for it in range(IT):
                        wgT = w_pool.tile([128, HT, 128], dt.bfloat16,
                                          tag="wstrip", name=f"wgT_{tb}_{it}")
                        wuT = w_pool.tile([128, HT, 128], dt.bfloat16,
                                          tag="wstrip", name=f"wuT_{tb}_{it}")
                        nc.sync.dma_start(
                            wgT.rearrange("p a b -> p (a b)"), gT_dram[it][:])
                        nc.gpsimd.dma_start(
                            wuT.rearrange("p a b -> p (a b)"), uT_dram[it][:])

                        gps = ps_gu.tile([128, TB], dt.float32, tag="gps",
                                         name=f"gps_{tb}_{it}")
                        ups = ps_gu.tile([128, TB], dt.float32, tag="ups",
                                         name=f"ups_{tb}_{it}")
                        for n in range(TB // 512):
                            nsl = bass.ts(n, 512)
                            for ht in range(HT):
                                nc.tensor.matmul(
                                    gps[:, nsl],
                                    wgT[:, ht, :],
                                    xT[:, ht, nsl],
                                    start=(ht == 0), stop=(ht == HT - 1),
                                )
                            for ht in range(HT):
                                last_up_mm = nc.tensor.matmul(
                                    ups[:, nsl],
                                    wuT[:, ht, :],
                                    xT[:, ht, nsl],
                                    start=(ht == 0), stop=(ht == HT - 1),
                                )
                        sil = a_pool.tile([128, TB], dt.bfloat16, tag="sil",
                                          name=f"sil_{tb}_{it}")
                        nc.scalar.activation(sil[:], gps[:],
                                             mybir.ActivationFunctionType.Silu)
                        nc.vector.tensor_tensor(h3[:, it, :], sil[:], ups[:],
                                                op=Alu.mult)

                if tb == 0:
                    # down xbar transposes go on the sync ring after tb0's gate
                    # strip loads so those aren't FIFO-blocked behind them.
                    emit_transpose_d()

                part = dram.tile([H, TB], dt.bfloat16, tag=f"part{tb}",
                                 name=f"part{tb}")
                first_down_mm = [None]
                with nc.named_scope(f"down_{tb}"):
                    for q in range(NQ):
                        wdT = d_pool.tile([128, IT, QH], dt.bfloat16,
                                          tag="dstrip", name=f"wdT_{tb}_{q}")
                        nc.gpsimd.dma_start(
                            wdT.rearrange("p a b -> p (a b)"), dT_dram[q][:])
                        for ho in range(QH // 128):
                            dps = ps_d.tile([128, TB], dt.float32, tag="dps",
                                            name=f"dps_{tb}_{q}_{ho}")
                            for n in range(TB // 512):
                                nsl = bass.ts(n, 512)
                                for it in range(IT):
                                    mm = nc.tensor.matmul(
                                        dps[:, nsl],
                                        wdT[:, it, ho * 128:(ho + 1) * 128],
                                        h3[:, it, nsl],
                                        start=(it == 0), stop=(it == IT - 1),
                                    )
                                    if first_down_mm[0] is None:
                                        first_down_mm[0] = mm
                                        # keep the down block after this tb's
                                        # gate/up matmuls in the PE stream
                                        add_dep_helper(
                                            mm.ins, last_up_mm.ins, sync=False,
                                            reason="down after gateup")
                            ob = o_pool.tile([128, TB], dt.bfloat16, tag="ob",
                                             name=f"ob_{tb}_{q}_{ho}")
                            nc.scalar.copy(ob[:], dps[:])
                            nc.gpsimd.dma_start(
                                part[(q * (QH // 128) + ho) * 128:
                                     (q * (QH // 128) + ho + 1) * 128, :],
                                ob[:],
                            )

                rs_o = dram.tile([H // N_CORES, TB], dt.bfloat16, tag=f"rs{tb}",
                                 name=f"rs{tb}")
                nc.gpsimd.collective_compute(
                    "ReduceScatter",
                    Alu.add,
                    replica_groups=[list(range(N_CORES))],
                    ins=[part.opt()],
                    outs=[rs_o.opt()],
                )
                nc.gpsimd.dma_start(outT[:, tb * TB:(tb + 1) * TB], rs_o[:])

    nc.compile()
    return nc


def _unpack_codes(Wq):
    """int32 [out, in/2] holding 0..255 byte values -> uint8 codes [out, in].
    Column 2j is the high nibble of byte j, column 2j+1 the low nibble."""
    b = Wq.astype(np.uint8)
    codes = np.empty((Wq.shape[0], Wq.shape[1] * 2), np.uint8)
    codes[:, 0::2] = (b >> 4) & 0xF
    codes[:, 1::2] = b & 0xF
    return codes


def _pad_rows(a, n):
    if a.shape[0] == n:
        return np.ascontiguousarray(a)
    pad = np.zeros((n - a.shape[0],) + a.shape[1:], a.dtype)
    return np.ascontiguousarray(np.concatenate([a, pad], axis=0))


def _pad_cols(a, n):
    if a.shape[1] == n:
        return np.ascontiguousarray(a)
    pad = np.zeros((a.shape[0], n - a.shape[1]), a.dtype)
    return np.ascontiguousarray(np.concatenate([a, pad], axis=1))


def kernel(x, gate_Wq, up_Wq, down_Wq, gate_scale, gate_zero,
           up_scale, up_zero, down_scale, down_zero):
    global LAST_RESULTS

    x2 = np.ascontiguousarray(np.asarray(x, np.float32).reshape(T, H))
    g_codes = _unpack_codes(np.asarray(gate_Wq))
    u_codes = _unpack_codes(np.asarray(up_Wq))
    d_codes = _unpack_codes(np.asarray(down_Wq))

    starts = np.cumsum([0] + CORE_SIZES)
    in_maps = []
    for c in range(N_CORES):
        lo, hi = int(starts[c]), int(starts[c + 1])
        glo, ghi = lo // G, hi // G
        in_maps.append({
            "x": x2,
            "gc": _pad_rows(g_codes[lo:hi], ISL),
            "uc": _pad_rows(u_codes[lo:hi], ISL),
            "dc": _pad_cols(d_codes[:, lo:hi], ISL),
            "gs": _pad_rows(np.asarray(gate_scale, np.float32)[lo:hi], ISL),
            "gz": _pad_rows(np.asarray(gate_zero, np.float32)[lo:hi], ISL),
            "us": _pad_rows(np.asarray(up_scale, np.float32)[lo:hi], ISL),
            "uz": _pad_rows(np.asarray(up_zero, np.float32)[lo:hi], ISL),
            "dsc": _pad_cols(np.asarray(down_scale, np.float32)[:, glo:ghi], DG),
            "dzr": _pad_cols(np.asarray(down_zero, np.float32)[:, glo:ghi], DG),
        })

    nc = _build()

    trace = os.environ.get("KERNEL_TRACE", "0") == "1"
    kw = {}
    if trace:
        kw = dict(trace=True, trace_cores=[0])
    res = bass_utils.run_bass_kernel_spmd(
        nc, in_maps, core_ids=list(range(N_CORES)), **kw)
    LAST_RESULTS = res

    out = np.empty((T, H), np.float32)
    shard = H // N_CORES
    for c in range(N_CORES):
        out[:, c * shard:(c + 1) * shard] = res.results[c]["outT"].T
    return out.reshape(B, S, H)


# revision 15
# speedup vs baseline: 1.1458x; 1.0589x over previous
"""Trainium2 Bass kernel for an int4-quantized DeepseekMLP (gate/up/down + SiLU).

Strategy (8 NeuronCores, tensor-parallel over the intermediate dim):
  - Each core owns a slice of the 11008 intermediate rows (6x1408 + 2x1280,
    padded to a uniform 1408 with zero-scale rows so all cores run one NEFF).
  - On device, per core:
      * x [4096, 4096] fp32 is cast to bf16 (DRAM->DRAM cast DMA, column
        chunks so the x^T transposes can start early).
      * int4 codes (host-unpacked to uint8) are dequantized on the DVE with two
        tensor_tensor ops per 128-row tile (subtract zero, multiply scale) using
        step-0 broadcast APs over the per-group scale/zero vectors.
      * Dequantized weights are transposed ONCE through the DMA xbar into a
        W^T DRAM scratch; the main loop re-reads them with plain DMAs.
      * The three matmuls run on the PE with everything in transposed layout
        (contraction dim on partitions). g^T/u^T accumulate in PSUM; SiLU runs
        on the scalar engine straight from PSUM; h^T = silu(g^T)*u^T on the
        DVE feeds the down matmul.
      * Partial down outputs (out^T) are ReduceScattered (bf16) over the 8
        cores along the output-feature dim, one collective per token block.
  - Host reassembles the full [4, 1024, 4096] fp32 output from the 8 shards.

HWDGE ring discipline (FIFO per issuing engine, so emission order == service
order): the ACT ring carries x^T xbar transposes + up-strip loads; the SP
(sync) ring carries weight-dequant xbar transposes + gate/down strip loads,
with the down transposes emitted after gateup_0 so tb0's gate strips aren't
stuck behind them. Everything else (codes, stores, casts, collectives) runs
on the gpsimd SWDGE path.
"""

import os

import numpy as np

import concourse.bass as bass
import concourse.mybir as mybir
import concourse.tile as tile
from concourse.tile import add_dep_helper
from concourse import bacc
import concourse.bass_utils as bass_utils

N_CORES = 8
B, S = 4, 1024
T = B * S            # 4096 tokens
H = 4096             # hidden
INTER = 11008
ISL = 1408           # per-core inter slice (padded)
G = 64               # quant group size
TB = 1024            # token block
NTB = T // TB        # 4
HT = H // 128        # 32 k-tiles for gate/up
IT = ISL // 128      # 11 i-tiles
NGH = H // G         # 64 groups along hidden (gate/up)
DG = ISL // G        # 22 groups along inter slice (down)
QH = 256             # down ho-slab height
NQ = H // QH         # 16 slabs

CORE_SIZES = [1408] * 6 + [1280] * 2

dt = mybir.dt
Alu = mybir.AluOpType

LAST_RESULTS = None


def _build():
    nc = bacc.Bacc("TRN2", target_bir_lowering=False, debug=False,
                   num_devices=N_CORES)

    x = nc.dram_tensor("x", [T, H], dt.float32, kind="ExternalInput")
    gc = nc.dram_tensor("gc", [ISL, H], dt.uint8, kind="ExternalInput")
    uc = nc.dram_tensor("uc", [ISL, H], dt.uint8, kind="ExternalInput")
    dc = nc.dram_tensor("dc", [H, ISL], dt.uint8, kind="ExternalInput")
    gs = nc.dram_tensor("gs", [ISL, NGH], dt.float32, kind="ExternalInput")
    gz = nc.dram_tensor("gz", [ISL, NGH], dt.float32, kind="ExternalInput")
    us = nc.dram_tensor("us", [ISL, NGH], dt.float32, kind="ExternalInput")
    uz = nc.dram_tensor("uz", [ISL, NGH], dt.float32, kind="ExternalInput")
    dsc = nc.dram_tensor("dsc", [H, DG], dt.float32, kind="ExternalInput")
    dzr = nc.dram_tensor("dzr", [H, DG], dt.float32, kind="ExternalInput")
    outT = nc.dram_tensor("outT", [H // N_CORES, T], dt.float32,
                          kind="ExternalOutput")

    with tile.TileContext(nc) as tc:
        with (
            tc.tile_pool(name="dram", bufs=1, space="DRAM") as dram,
            tc.tile_pool(name="xt", bufs=1) as xt_pool,
            tc.tile_pool(name="hp", bufs=2) as h_pool,
            tc.tile_pool(name="stage", bufs=3) as st_pool,   # dequant/xprep staging
            tc.tile_pool(name="wstream", bufs=5) as w_pool,  # main-loop gu strips
            tc.tile_pool(name="dstream", bufs=2) as d_pool,  # main-loop down strips
            tc.tile_pool(name="codes", bufs=2) as c_pool,
            tc.tile_pool(name="sz", bufs=4) as sz_pool,
            tc.tile_pool(name="act", bufs=2) as a_pool,
            tc.tile_pool(name="ob", bufs=2) as o_pool,
            tc.tile_pool(name="psgu", bufs=1, space="PSUM") as ps_gu,
            tc.tile_pool(name="psd", bufs=2, space="PSUM") as ps_d,
        ):
            # ---- x^T prep: load fp32 rows, cast to bf16 on the DVE, then
            # xbar-transpose SBUF->SBUF straight into the xT tile.
            def make_xT(tb):
                xT = xt_pool.tile([128, HT, TB], dt.bfloat16, tag="xT",
                                  name=f"xT_{tb}")
                with nc.named_scope(f"xT_{tb}"):
                    for rt in range(TB // 128):
                        rows = slice(tb * TB + rt * 128, tb * TB + (rt + 1) * 128)
                        for hf in range(2):
                            hsl = bass.ts(hf, H // 2)
                            xrow = st_pool.tile([128, H // 2], dt.float32,
                                                tag="stage",
                                                name=f"xrow_{tb}_{rt}_{hf}")
                            nc.scalar.dma_start(xrow[:], x[rows, hsl])
                            xrbf = st_pool.tile([128, H // 2], dt.bfloat16,
                                                tag="stage",
                                                name=f"xrbf_{tb}_{rt}_{hf}")
                            nc.vector.tensor_copy(xrbf[:], xrow[:])
                            nc.scalar.dma_start(
                                xT[:, hf * (HT // 2):(hf + 1) * (HT // 2),
                                   rt * 128:(rt + 1) * 128],
                                xrbf[:],
                                transpose=True,
                            )
                return xT

            # ---- dequant weights + transpose-once into W^T DRAM
            def dequant_rows(codes_dram, s_dram, z_dram, it, width, ngroups, tag):
                """One 128-row tile: (codes - zero) * scale with broadcast APs."""
                cs = c_pool.tile([128, width], dt.uint8, tag="codes",
                                 name=f"cs_{tag}")
                nc.gpsimd.dma_start(cs[:], codes_dram[it * 128:(it + 1) * 128, :])
                ssb = sz_pool.tile([128, ngroups], dt.float32, tag="ssb",
                                   name=f"ssb_{tag}")
                zsb = sz_pool.tile([128, ngroups], dt.float32, tag="zsb",
                                   name=f"zsb_{tag}")
                nc.gpsimd.dma_start(ssb[:], s_dram[it * 128:(it + 1) * 128, :])
                nc.gpsimd.dma_start(zsb[:], z_dram[it * 128:(it + 1) * 128, :])
                tmp = st_pool.tile([128, width], dt.bfloat16, tag="stage",
                                   name=f"tmp_{tag}")
                wb = st_pool.tile([128, width], dt.bfloat16, tag="stage",
                                  name=f"wb_{tag}")
                nc.vector.tensor_tensor(
                    tmp.rearrange("p (g k) -> p g k", k=G),
                    cs.rearrange("p (g k) -> p g k", k=G),
                    zsb[:, :, None].broadcast_to([128, ngroups, G]),
                    op=Alu.subtract,
                )
                nc.vector.tensor_tensor(
                    wb.rearrange("p (g k) -> p g k", k=G),
                    tmp.rearrange("p (g k) -> p g k", k=G),
                    ssb[:, :, None].broadcast_to([128, ngroups, G]),
                    op=Alu.mult,
                )
                return wb

            # gate/up: dequant [128, H] rows, xbar-transpose straight from SBUF
            # into a [128, HT, 128] strip, store to W^T DRAM for plain re-reads.
            gT_dram, uT_dram = [], []
            gT_sb, uT_sb = [], []

            def dequant_gu_tile(it, nm, codes_d, s_d, z_d, lst, sb_lst):
                wb = dequant_rows(codes_d, s_d, z_d, it, H, NGH, f"{nm}{it}")
                wTs = w_pool.tile([128, HT, 128], dt.bfloat16,
                                   tag="wstrip", name=f"wTs_{nm}{it}")
                nc.sync.dma_start(wTs[:, :, :], wb[:], transpose=True)
                wT_d = dram.tile([128, HT * 128], dt.bfloat16,
                                 tag=f"{nm}T{it}", name=f"{nm}T{it}")
                nc.gpsimd.dma_start(wT_d[:], wTs.rearrange("p a b -> p (a b)"))
                lst.append(wT_d)
                sb_lst.append(wTs)

            # it=0 strips first so the PE can start as soon as x^T lands
            with nc.named_scope("dequant_gu"):
                dequant_gu_tile(0, "g", gc, gs, gz, gT_dram, gT_sb)
                dequant_gu_tile(0, "u", uc, us, uz, uT_dram, uT_sb)

            xT0 = make_xT(0)

            with nc.named_scope("dequant_gu"):
                for it in range(1, IT):
                    dequant_gu_tile(it, "g", gc, gs, gz, gT_dram, gT_sb)
                    dequant_gu_tile(it, "u", uc, us, uz, uT_dram, uT_sb)

            # down: dequant [128, ISL] rows into slab DRAM tiles (natural layout)
            d_nat = [dram.tile([QH, ISL], dt.bfloat16, tag=f"dnat{q}",
                               name=f"dnat{q}")
                     for q in range(NQ)]
            with nc.named_scope("dequant_d"):
                for ot in range(H // 128):
                    wb = dequant_rows(dc, dsc, dzr, ot, ISL, DG, f"d{ot}")
                    q, r = divmod(ot, QH // 128)
                    nc.gpsimd.dma_start(d_nat[q][r * 128:(r + 1) * 128, :], wb[:])

            dT_dram = [dram.tile([128, IT * QH], dt.bfloat16, tag=f"dT{q}",
                                 name=f"dT{q}")
                       for q in range(NQ)]

            def emit_transpose_d():
                # one 3D-dest xbar transpose per down slab into W^T DRAM:
                # dest[p, it, j] = d_nat[q][j, it*128 + p]
                with nc.named_scope("transpose_d"):
                    for q in range(NQ):
                        dTs = st_pool.tile([128, IT, QH], dt.bfloat16,
                                           tag="stage", name=f"dTs_{q}")
                        nc.sync.dma_start(dTs[:, :, :], d_nat[q][:, :],
                                          transpose=True)
                        nc.gpsimd.dma_start(
                            dT_dram[q][:], dTs.rearrange("p a b -> p (a b)"))

            # ---- main loop over token blocks
            for tb in range(NTB):
                xT = xT0 if tb == 0 else make_xT(tb)

                h3 = h_pool.tile([128, IT, TB], dt.bfloat16, tag="h3",
                                 name=f"h3_{tb}")
                with nc.named_scope(f"gateup_{tb}"):
                    for it in range(IT):
                        if tb == 0:
                            # read the dequant xbar output directly from SBUF
                            wgT, wuT = gT_sb[it], uT_sb[it]
                        else:
                            wgT = w_pool.tile([128, HT, 128], dt.bfloat16,
                                              tag="wstrip",
                                              name=f"wgT_{tb}_{it}")
                            wuT = w_pool.tile([128, HT, 128], dt.bfloat16,
                                              tag="wstrip",
                                              name=f"wuT_{tb}_{it}")
                            nc.sync.dma_start(
                                wgT.rearrange("p a b -> p (a b)"),
                                gT_dram[it][:])
                            nc.gpsimd.dma_start(
                                wuT.rearrange("p a b -> p (a b)"),
                                uT_dram[it][:])

                        gps = ps_gu.tile([128, TB], dt.float32, tag="gps",
                                         name=f"gps_{tb}_{it}")
                        ups = ps_gu.tile([128, TB], dt.float32, tag="ups",
                                         name=f"ups_{tb}_{it}")
                        for n in range(TB // 512):
                            nsl = bass.ts(n, 512)
                            for ht in range(HT):
                                nc.tensor.matmul(
                                    gps[:, nsl],
                                    wgT[:, ht, :],
                                    xT[:, ht, nsl],
                                    start=(ht == 0), stop=(ht == HT - 1),
                                )
                            for ht in range(HT):
                                last_up_mm = nc.tensor.matmul(
                                    ups[:, nsl],
                                    wuT[:, ht, :],
                                    xT[:, ht, nsl],
                                    start=(ht == 0), stop=(ht == HT - 1),
                                )
                        sil = a_pool.tile([128, TB], dt.bfloat16, tag="sil",
                                          name=f"sil_{tb}_{it}")
                        nc.scalar.activation(sil[:], gps[:],
                                             mybir.ActivationFunctionType.Silu)
                        nc.vector.tensor_tensor(h3[:, it, :], sil[:], ups[:],
                                                op=Alu.mult)

                if tb == 0:
                    # down xbar transposes go on the sync ring after tb0's gate
                    # strip loads so those aren't FIFO-blocked behind them.
                    emit_transpose_d()

                part = dram.tile([H, TB], dt.bfloat16, tag=f"part{tb}",
                                 name=f"part{tb}")
                first_down_mm = [None]
                with nc.named_scope(f"down_{tb}"):
                    for q in range(NQ):
                        wdT = d_pool.tile([128, IT, QH], dt.bfloat16,
                                          tag="dstrip", name=f"wdT_{tb}_{q}")
                        nc.gpsimd.dma_start(
                            wdT.rearrange("p a b -> p (a b)"), dT_dram[q][:])
                        for ho in range(QH // 128):
                            dps = ps_d.tile([128, TB], dt.float32, tag="dps",
                                            name=f"dps_{tb}_{q}_{ho}")
                            for n in range(TB // 512):
                                nsl = bass.ts(n, 512)
                                for it in range(IT):
                                    mm = nc.tensor.matmul(
                                        dps[:, nsl],
                                        wdT[:, it, ho * 128:(ho + 1) * 128],
                                        h3[:, it, nsl],
                                        start=(it == 0), stop=(it == IT - 1),
                                    )
                                    if first_down_mm[0] is None:
                                        first_down_mm[0] = mm
                                        # keep the down block after this tb's
                                        # gate/up matmuls in the PE stream
                                        add_dep_helper(
                                            mm.ins, last_up_mm.ins, sync=False,
                                            reason="down after gateup")
                            ob = o_pool.tile([128, TB], dt.bfloat16, tag="ob",
                                             name=f"ob_{tb}_{q}_{ho}")
                            nc.scalar.copy(ob[:], dps[:])
                            nc.gpsimd.dma_start(
                                part[(q * (QH // 128) + ho) * 128:
                                     (q * (QH // 128) + ho + 1) * 128, :],
                                ob[:],
                            )

                rs_o = dram.tile([H // N_CORES, TB], dt.bfloat16, tag=f"rs{tb}",
                                 name=f"rs{tb}")
                nc.gpsimd.collective_compute(
                    "ReduceScatter",
                    Alu.add,
                    replica_groups=[list(range(N_CORES))],
                    ins=[part.opt()],
                    outs=[rs_o.opt()],
                )
                nc.gpsimd.dma_start(outT[:, tb * TB:(tb + 1) * TB], rs_o[:])

    nc.compile()
    return nc


def _unpack_codes(Wq):
    """int32 [out, in/2] holding 0..255 byte values -> uint8 codes [out, in].
    Column 2j is the high nibble of byte j, column 2j+1 the low nibble."""
    b = Wq.astype(np.uint8)
    codes = np.empty((Wq.shape[0], Wq.shape[1] * 2), np.uint8)
    codes[:, 0::2] = (b >> 4) & 0xF
    codes[:, 1::2] = b & 0xF
    return codes


def _pad_rows(a, n):
    if a.shape[0] == n:
        return np.ascontiguousarray(a)
    pad = np.zeros((n - a.shape[0],) + a.shape[1:], a.dtype)
    return np.ascontiguousarray(np.concatenate([a, pad], axis=0))


def _pad_cols(a, n):
    if a.shape[1] == n:
        return np.ascontiguousarray(a)
    pad = np.zeros((a.shape[0], n - a.shape[1]), a.dtype)
    return np.ascontiguousarray(np.concatenate([a, pad], axis=1))


def kernel(x, gate_Wq, up_Wq, down_Wq, gate_scale, gate_zero,
           up_scale, up_zero, down_scale, down_zero):
    global LAST_RESULTS

    x2 = np.ascontiguousarray(np.asarray(x, np.float32).reshape(T, H))
    g_codes = _unpack_codes(np.asarray(gate_Wq))
    u_codes = _unpack_codes(np.asarray(up_Wq))
    d_codes = _unpack_codes(np.asarray(down_Wq))

    starts = np.cumsum([0] + CORE_SIZES)
    in_maps = []
    for c in range(N_CORES):
        lo, hi = int(starts[c]), int(starts[c + 1])
        glo, ghi = lo // G, hi // G
        in_maps.append({
            "x": x2,
            "gc": _pad_rows(g_codes[lo:hi], ISL),
            "uc": _pad_rows(u_codes[lo:hi], ISL),
            "dc": _pad_cols(d_codes[:, lo:hi], ISL),
            "gs": _pad_rows(np.asarray(gate_scale, np.float32)[lo:hi], ISL),
            "gz": _pad_rows(np.asarray(gate_zero, np.float32)[lo:hi], ISL),
            "us": _pad_rows(np.asarray(up_scale, np.float32)[lo:hi], ISL),
            "uz": _pad_rows(np.asarray(up_zero, np.float32)[lo:hi], ISL),
            "dsc": _pad_cols(np.asarray(down_scale, np.float32)[:, glo:ghi], DG),
            "dzr": _pad_cols(np.asarray(down_zero, np.float32)[:, glo:ghi], DG),
        })

    nc = _build()

    trace = os.environ.get("KERNEL_TRACE", "0") == "1"
    kw = {}
    if trace:
        kw = dict(trace=True, trace_cores=[0])
    res = bass_utils.run_bass_kernel_spmd(
        nc, in_maps, core_ids=list(range(N_CORES)), **kw)
    LAST_RESULTS = res

    out = np.empty((T, H), np.float32)
    shard = H // N_CORES
    for c in range(N_CORES):
        out[:, c * shard:(c + 1) * shard] = res.results[c]["outT"].T
    return out.reshape(B, S, H)
